# revision 3
# baseline (speedup 1.0000x reference)
"""Two-layer GAT (PyG GATConv semantics) on 8 Trainium2 NeuronCores — v3.

Gather strategy: batched SWDGE dma_gather (~0.34ns/row descriptor cost)
instead of v1's per-slot indirect DMAs (~1us fixed cost each, ~4000 of
them = 4ms).

dma_gather constraints shape the whole layout:
  - int16 row indices -> the node table is cut into NW=4 windows; each
    gather call addresses one window (idx relative to window base).
  - Rows must be a multiple of 256B -> layer-1 rows pack h as fp8 (128B)
    + asn as bf16 (16B); layer-2 rows pack h2 bf16 (128B) + asn2 (2B).
  - Row i of a call lands at SBUF (partition i%128, slot i//128), and all
    indices must be valid -> padding slots point at a per-window sentinel
    row (h=0, asn=-60 so exp(lrelu(...)) ~ 0).

Node placement (host side): a greedy + refined 4-coloring balances each
dst's in-edge sources across the 4 windows (cuts slot padding from ~2.0x
to ~1.6x); within each (core, window) block nodes are in-degree sorted
into batches of 128 so the per-batch max degree (column count) is small.

Per chunk of batches, slab columns are window-major: [w0 blocks | w1 |
w2 | w3], one dma_gather per (chunk, window). Softmax denominators and
messages are per-batch reductions assembled from 4 window partials.

AllGather chunks are window-aligned and interleaved with compute so
edge-phase gathers start as soon as their window's table section lands.
"""

import sys

for _p in ("/opt/trn_rl_repo",):
    if _p not in sys.path:
        sys.path.insert(0, _p)

import numpy as np

from concourse import bacc, bass, mybir, tile
from concourse.bass_utils import run_bass_kernel_spmd
from concourse.masks import make_identity

F32 = mybir.dt.float32
BF16 = mybir.dt.bfloat16
F8 = mybir.dt.float8e4
I16 = mybir.dt.int16

N_CORES = 8
P = 128
NW = 4
NEG_SLOPE = 0.2
SENT_ASN = -60.0
ROWB = 256          # table row bytes
RE = ROWB // 2      # row length in bf16 elements
S_CHUNK = 192       # max slab columns per chunk
BUILD_VARIANT = "full"   # "p1" | "e1" | "full" (phase bisection)
QUEUE_RR = True          # round-robin dma_gather queue_num across 0..3
SIM_ZERO_PAD = False     # zero table pad columns (sim NaN checker only)


class Plan:
    pass


# ---------------------------------------------------------------------------
# Host-side graph planning
# ---------------------------------------------------------------------------

def _color_windows(src, dst, n_nodes, shard, winb):
    order = np.argsort(src, kind="stable")
    d_sorted = dst[order]
    starts = np.searchsorted(src[order], np.arange(n_nodes + 1))

    cnt = np.zeros((n_nodes, NW), dtype=np.int32)
    cap = np.zeros((N_CORES, NW), dtype=np.int64)
    capmax = winb - 1
    color = np.zeros(n_nodes, dtype=np.int8)

    rng = np.random.default_rng(0)
    B = 512

    def _sweep(first):
        for i in range(0, n_nodes, B):
            us = proc[i:i + B]
            counts = starts[us + 1] - starts[us]
            rows = np.repeat(np.arange(len(us)), counts)
            flat = np.concatenate(
                [d_sorted[starts[u]:starts[u + 1]] for u in us])
            score = np.zeros((len(us), NW), dtype=np.float64)
            np.add.at(score, rows, cnt[flat])
            cores = us // shard
            w0 = color[us]
            if not first:
                score[np.arange(len(us)), w0] -= counts
            full = cap[cores] >= capmax
            if not first:
                full[np.arange(len(us)), w0] = False
            score[full] = np.inf
            w = np.argmin(score, axis=1).astype(np.int8)
            if first:
                color[us] = w
                np.add.at(cap, (cores, w), 1)
                np.add.at(cnt, (flat, w[rows]), 1)
            else:
                moved = w != w0
                um = us[moved]
                color[um] = w[moved]
                np.add.at(cap, (um // shard, w[moved]), 1)
                np.add.at(cap, (um // shard, w0[moved]), -1)
                mrows = moved[rows]
                np.add.at(cnt, (flat[mrows], w[rows][mrows]), 1)
                np.add.at(cnt, (flat[mrows], w0[rows][mrows]), -1)

    proc = rng.permutation(n_nodes)
    _sweep(True)
    # capacity repair: block-greedy can overshoot capmax slightly
    for c in range(N_CORES):
        ids = np.arange(c * shard, (c + 1) * shard)
        for wv in range(NW):
            over = int(cap[c, wv]) - capmax
            if over <= 0:
                continue
            sel = ids[color[ids] == wv][:over]
            for u in sel:
                w2 = int(np.argmin(cap[c]))
                ds = d_sorted[starts[u]:starts[u + 1]]
                cnt[ds, wv] -= 1
                cnt[ds, w2] += 1
                cap[c, wv] -= 1
                cap[c, w2] += 1
                color[u] = w2
    for _ in range(2):
        proc = rng.permutation(n_nodes)
        _sweep(False)
    for c in range(N_CORES):
        assert cap[c].max() <= capmax
    return color, cnt


def build_plan(edge_index, n_nodes):
    assert n_nodes % N_CORES == 0
    shard = n_nodes // N_CORES
    winb = -(-shard // (NW * P)) * P       # rows per (core, window) block
    pos = NW * winb                        # padded positions per core
    nbq = winb // P                        # batches per quarter
    nb = NW * nbq
    win = N_CORES * winb                   # rows per window in full table
    sent_local = winb - 1
    pg = 5 if nbq % 5 == 0 else 1          # staging group (divides nbq)

    src_all = np.concatenate(
        [edge_index[0].astype(np.int64), np.arange(n_nodes, dtype=np.int64)])
    dst_all = np.concatenate(
        [edge_index[1].astype(np.int64), np.arange(n_nodes, dtype=np.int64)])

    color, cnt = _color_windows(src_all, dst_all, n_nodes, shard, winb)
    owner = dst_all // shard

    posq_cores, deg_cores, edges_cores = [], [], []
    for c in range(N_CORES):
        m = owner == c
        es = src_all[m]
        ed = dst_all[m] - c * shard
        o = np.argsort(ed, kind="stable")
        es, ed = es[o], ed[o]
        deg = np.bincount(ed, minlength=shard)
        assert deg.min() >= 1
        posq = np.empty(shard, dtype=np.int64)
        col_local = color[c * shard:(c + 1) * shard]
        for k in range(NW):
            sel = np.where(col_local == k)[0]
            assert len(sel) <= winb - 1
            o2 = np.argsort(-deg[sel], kind="stable")
            posq[sel[o2]] = k * winb + np.arange(len(sel))
        posq_cores.append(posq)
        deg_cores.append(deg)
        edges_cores.append((es, ed))

    posmap = np.empty(n_nodes, dtype=np.int64)
    for c in range(N_CORES):
        q = posq_cores[c]
        k = q // winb
        posmap[c * shard:(c + 1) * shard] = \
            k * win + c * winb + (q - k * winb)

    # shared per-(batch, window) column counts (max over cores+partitions)
    Lbw = np.zeros((nb, NW), dtype=np.int64)
    for c in range(N_CORES):
        cw = np.zeros((pos, NW), dtype=np.int64)
        cw[posq_cores[c]] = cnt[c * shard:(c + 1) * shard]
        Lbw = np.maximum(Lbw, cw.reshape(nb, P, NW).max(axis=1))
    Lbw = np.maximum(Lbw, 1)

    # chunks of batches (never crossing quarter boundaries)
    Lb = Lbw.sum(axis=1)
    chunks = []
    b = 0
    while b < nb:
        e = b
        s = 0
        while (e < nb and e // nbq == b // nbq and s + Lb[e] <= S_CHUNK):
            s += Lb[e]
            e += 1
        if e == b:
            e = b + 1
        chunks.append((b, e))
        b = e

    chunk_info = []
    gidx_cols = 0
    boff_full = np.zeros((nb, NW), dtype=np.int64)   # col of (b, w) block
    gcol_of_b = np.zeros(nb, dtype=np.int64)
    for (b0, b1) in chunks:
        Sw = Lbw[b0:b1].sum(axis=0)
        S = int(Sw.sum())
        wstart = np.concatenate([[0], np.cumsum(Sw)])[:NW]
        for wv in range(NW):
            boff_full[b0:b1, wv] = wstart[wv] + np.concatenate(
                [[0], np.cumsum(Lbw[b0:b1, wv])])[:b1 - b0]
        gcol_of_b[b0:b1] = gidx_cols
        ci = Plan()
        ci.b0, ci.b1, ci.S = b0, b1, S
        ci.Sw = Sw.astype(int)
        ci.wstart = wstart.astype(int)
        ci.gcol = gidx_cols
        gidx_cols += 8 * S
        chunk_info.append(ci)

    pc = []
    for c in range(N_CORES):
        es, ed = edges_cores[c]
        posq = posq_cores[c]

        erow = posmap[es]
        ew = erow // win
        eidx = erow - ew * win
        q = posq[ed]
        eb = q // P
        ep = q % P

        # rank within (dst, window) run
        o3 = np.lexsort((ew, ed))
        ed2, ew2, eidx2, eb2, ep2 = \
            ed[o3], ew[o3], eidx[o3], eb[o3], ep[o3]
        key = ed2 * NW + ew2
        first_of = np.r_[True, key[1:] != key[:-1]]
        run_id = np.cumsum(first_of) - 1
        run_start = np.where(first_of)[0]
        j_in_run = np.arange(len(ed2)) - run_start[run_id]

        col_e = boff_full[eb2, ew2] + j_in_run
        i_e = col_e * P + ep2
        gc = gcol_of_b[eb2] + i_e // 16

        gidx = np.full((16, gidx_cols), sent_local, dtype=np.int16)
        gidx[(i_e % 16).astype(np.int64), gc.astype(np.int64)] = \
            eidx2.astype(np.int16)

        pl = Plan()
        pl.gidx = np.tile(gidx, (8, 1))
        pl.posq = posq
        pc.append(pl)

    plan = Plan()
    plan.shard, plan.winb, plan.pos, plan.nbq, plan.nb = \
        shard, winb, pos, nbq, nb
    plan.win, plan.sent_local, plan.pg = win, sent_local, pg
    plan.Lbw, plan.chunks, plan.chunk_info = Lbw, chunks, chunk_info
    plan.gidx_cols = gidx_cols
    plan.cores = pc
    return plan


# ---------------------------------------------------------------------------
# Device program
# ---------------------------------------------------------------------------

def build_program(plan, f_in, h1, c1, ncls):
    F1 = h1 * c1
    GC = plan.gidx_cols
    POS, WINB, WIN = plan.pos, plan.winb, plan.win
    NB, NBQ, PG = plan.nb, plan.nbq, plan.pg
    SENT = plan.sent_local
    Lbw, chunk_info = plan.Lbw, plan.chunk_info

    nc = bacc.Bacc(target_bir_lowering=False, debug=False,
                   num_devices=N_CORES, num_swdge_queues=4)

    xT = nc.declare_dram_parameter("xT", [f_in, POS], BF16, isOutput=False)
    W1a = nc.declare_dram_parameter("W1a", [f_in, F1 + 2 * h1], BF16,
                                    isOutput=False)
    W2 = nc.declare_dram_parameter("W2", [F1, ncls], F32, isOutput=False)
    W2T = nc.declare_dram_parameter("W2T", [ncls, F1], F32, isOutput=False)
    A2 = nc.declare_dram_parameter("A2", [ncls, 2], F32, isOutput=False)
    b1r = nc.declare_dram_parameter("b1r", [1, F1], F32, isOutput=False)
    b2r = nc.declare_dram_parameter("b2r", [1, ncls], F32, isOutput=False)
    gidxD = nc.declare_dram_parameter("gidx", [P, GC], I16, isOutput=False)
    outD = nc.declare_dram_parameter("out", [POS, ncls], F32, isOutput=True)

    R1P = 64 + h1                 # packed t1 row elements (bf16)
    R2P = ncls + 1                # packed t2 row elements
    t1s = nc.dram_tensor("t1shard", [POS, R1P], BF16)
    t1p = nc.dram_tensor("t1pack", [N_CORES * POS, R1P], BF16,
                         addr_space="Shared")
    t1f = nc.dram_tensor("t1full", [N_CORES * POS, RE], BF16)
    t2s = nc.dram_tensor("t2shard", [POS, R2P], BF16)
    t2p = nc.dram_tensor("t2pack", [N_CORES * POS, R2P], BF16,
                         addr_space="Shared")
    t2f = nc.dram_tensor("t2full", [N_CORES * POS, RE], BF16)

    rg = [list(range(N_CORES))]

    from contextlib import ExitStack
    with tile.TileContext(nc) as tc, ExitStack() as ctx:
        const = ctx.enter_context(tc.tile_pool(name="const", bufs=1))
        sb = ctx.enter_context(tc.tile_pool(name="sb", bufs=2))
        ph1 = ctx.enter_context(tc.tile_pool(name="ph1", bufs=2))
        slabp = ctx.enter_context(tc.tile_pool(name="slabp", bufs=2))
        mp = ctx.enter_context(tc.tile_pool(name="mp", bufs=1))
        ixp = ctx.enter_context(tc.tile_pool(name="ixp", bufs=4))
        psc = ctx.enter_context(tc.tile_pool(name="psc", bufs=1,
                                             space="PSUM"))
        ps = ctx.enter_context(tc.tile_pool(name="ps", bufs=2, space="PSUM"))

        # ------------------- constants -------------------
        W1aug = const.tile([f_in, F1 + 2 * h1], BF16)
        nc.sync.dma_start(W1aug[:], W1a[:, :])
        W2sb = const.tile([F1, ncls], F32)
        nc.sync.dma_start(W2sb[:], W2[:, :])
        W2Tsb = const.tile([ncls, F1], F32)
        nc.sync.dma_start(W2Tsb[:], W2T[:, :])
        A2sb = const.tile([ncls, 2], F32)
        nc.sync.dma_start(A2sb[:], A2[:, :])
        b1row = const.tile([1, F1], F32)
        nc.sync.dma_start(b1row[:], b1r[:, :])
        b2row = const.tile([1, ncls], F32)
        nc.sync.dma_start(b2row[:], b2r[:, :])

        psB = psc.tile([F1, 2], F32, tag="ps_small")
        nc.tensor.matmul(psB[:], lhsT=W2Tsb[:], rhs=A2sb[:], start=True,
                         stop=True)
        W2aug = const.tile([F1, ncls + 2], BF16)
        nc.vector.tensor_copy(W2aug[:, 0:ncls], W2sb[:])
        nc.vector.tensor_copy(W2aug[:, ncls:ncls + 2], psB[:])

        ones1 = const.tile([1, P], F32)
        nc.vector.memset(ones1[:], 1.0)
        psb1 = psc.tile([P, F1], F32, tag="ps_bias")
        nc.tensor.matmul(psb1[:], lhsT=ones1[:], rhs=b1row[:], start=True,
                         stop=True)
        b1bc = const.tile([P, F1], F32)
        nc.vector.tensor_copy(b1bc[:], psb1[:])
        psb2 = psc.tile([P, ncls], F32, tag="ps_bias")
        nc.tensor.matmul(psb2[:], lhsT=ones1[:], rhs=b2row[:], start=True,
                         stop=True)
        b2bc = const.tile([P, ncls], F32)
        nc.vector.tensor_copy(b2bc[:], psb2[:])

        ident = const.tile([P, P], BF16)
        make_identity(nc, ident[:])

        # sentinel rows (element layout: [0:64]=h (fp8 bytes), 64..=asn)
        s1 = const.tile([1, 64 + h1], BF16)
        nc.vector.memset(s1[:], 0.0)
        nc.vector.memset(s1[:, 64:64 + h1], SENT_ASN)
        s2 = const.tile([1, ncls + 1], BF16)
        nc.vector.memset(s2[:], 0.0)
        nc.vector.memset(s2[:, ncls:ncls + 1], SENT_ASN)

        adn1sb = const.tile([P, NB, h1], BF16)
        adn2sb = const.tile([P, NB], BF16)

        if SIM_ZERO_PAD:
            zpad = const.tile([P, RE], BF16)
            nc.vector.memset(zpad[:], 0.0)
            ngr = N_CORES * POS // P
            for gi in range(ngr):
                nc.sync.dma_start(
                    t1f[gi * P:(gi + 1) * P, 64 + h1:RE],
                    zpad[:, 0:RE - 64 - h1])
                nc.sync.dma_start(
                    t2f[gi * P:(gi + 1) * P, ncls + 1:RE],
                    zpad[:, 0:RE - ncls - 1])

        # ---- phase 1 ----
        for g in range(NB // PG):
            b0 = g * PG
            xt = ph1.tile([f_in, PG * P], BF16, tag="xt")
            nc.sync.dma_start(xt[:], xT[:, b0 * P:(b0 + PG) * P])
            stage = ph1.tile([P, PG, 64 + h1], BF16, tag="stage")
            for bi in range(PG):
                b = b0 + bi
                p1 = ps.tile([P, F1 + 2 * h1], F32, tag="ps_p1")
                nc.tensor.matmul(p1[:], lhsT=xt[:, bi * P:(bi + 1) * P],
                                 rhs=W1aug[:], start=True, stop=True)
                nc.vector.tensor_copy(stage[:, bi, 0:F1 // 2].bitcast(F8),
                                      p1[:, 0:F1])
                nc.vector.tensor_copy(stage[:, bi, 64:64 + h1],
                                      p1[:, F1:F1 + h1])
                nc.vector.tensor_copy(adn1sb[:, b, :],
                                      p1[:, F1 + h1:F1 + 2 * h1])
            nc.scalar.dma_start(
                t1s[b0 * P:(b0 + PG) * P, :].rearrange(
                    "(b p) r -> p b r", p=P),
                stage[:])
            if (b0 + PG) % NBQ == 0 and "noag1" not in BUILD_VARIANT:
                k = (b0 + PG) // NBQ - 1
                nc.sync.dma_start(
                    t1s[k * WINB + SENT:k * WINB + SENT + 1, :], s1[:])
                nc.gpsimd.collective_compute(
                    "AllGather", mybir.AluOpType.bypass, replica_groups=rg,
                    ins=[t1s[k * WINB:(k + 1) * WINB, :].opt()],
                    outs=[t1p[k * WIN:(k + 1) * WIN, :].opt()])
                nc.sync.dma_start(t1f[k * WIN:(k + 1) * WIN, 0:R1P],
                                  t1p[k * WIN:(k + 1) * WIN, :])

        # ------------------- edge phases -------------------
        def edge_phase(layer):
            do_gather = "nogather" not in BUILD_VARIANT
            do_compute = "nocompute" not in BUILD_VARIANT
            table = t1f if layer == 1 else t2f
            H = h1 if layer == 1 else 1
            F = F1 if layer == 1 else ncls
            C = F // H
            AOFF = 64 if layer == 1 else ncls    # asn element offset in row
            bias = b1bc if layer == 1 else b2bc

            stctr = [None, 0, 0]  # tile, fill, first batch

            for ci in chunk_info:
                S = ci.S
                nbc = ci.b1 - ci.b0

                slab = slabp.tile([P, S_CHUNK, RE], BF16, tag="slab")
                for wv in range(NW):
                    Sw = int(ci.Sw[wv])
                    if Sw == 0 or not do_gather:
                        continue
                    ix = ixp.tile([P, 8 * S_CHUNK], I16, tag="ix")
                    g0 = ci.gcol + 8 * int(ci.wstart[wv])
                    nc.sync.dma_start(ix[:, 0:8 * Sw],
                                      gidxD[:, g0:g0 + 8 * Sw])
                    nc.gpsimd.dma_gather(
                        out_ap=slab[:,
                                    int(ci.wstart[wv]):
                                    int(ci.wstart[wv]) + Sw, :],
                        in_ap=table[wv * WIN:(wv + 1) * WIN, :],
                        idxs_ap=ix[:, 0:8 * Sw],
                        num_idxs=P * Sw, num_idxs_reg=P * Sw,
                        elem_size=RE, single_packet=False)

                if not do_compute:
                    continue
                adnx = sb.tile([P, S_CHUNK, h1], BF16, tag="adnx")
                for bi in range(nbc):
                    b = ci.b0 + bi
                    for wv in range(NW):
                        L = int(Lbw[b, wv])
                        o = int(ci.wstart[wv] + Lbw[ci.b0:b, wv].sum())
                        if layer == 1:
                            srcv = adn1sb[:, b:b + 1, :] \
                                .broadcast_to([P, L, H])
                        else:
                            srcv = adn2sb[:, b:b + 1].unsqueeze(2) \
                                .broadcast_to([P, L, H])
                        nc.vector.tensor_copy(adnx[:, o:o + L, 0:H], srcv)

                e = sb.tile([P, S_CHUNK, h1], F32, tag="e")
                ev = e[:, 0:S, 0:H]
                nc.vector.tensor_tensor(
                    out=ev, in0=slab[:, 0:S, AOFF:AOFF + H],
                    in1=adnx[:, 0:S, 0:H], op=mybir.AluOpType.add)
                nc.vector.scalar_tensor_tensor(
                    out=ev, in0=ev, scalar=NEG_SLOPE, in1=ev,
                    op0=mybir.AluOpType.mult, op1=mybir.AluOpType.max)
                ee = sb.tile([P, S_CHUNK, h1], BF16, tag="ee")
                nc.scalar.activation(ee[:, 0:S, 0:H], ev,
                                     mybir.ActivationFunctionType.Exp)

                m = mp.tile([P, F1, S_CHUNK], BF16, tag="m")
                if layer == 1:
                    m_v = m[:, :, 0:S].rearrange("p (h c) l -> p h c l",
                                                 h=H)
                    h_v = slab[:, 0:S, 0:F1 // 2].bitcast(F8) \
                        .rearrange("p l (h c) -> p h c l", h=H)
                    ee_v = ee[:, 0:S, 0:H].rearrange("p l h -> p h l") \
                        .unsqueeze(2).broadcast_to([P, H, C, S])
                else:
                    m_v = m[:, 0:F, 0:S]
                    h_v = slab[:, 0:S, 0:ncls].rearrange("p l c -> p c l")
                    ee_v = ee[:, 0:S, 0:1].rearrange("p l h -> p h l") \
                        .broadcast_to([P, C, S])
                nc.any.tensor_tensor(out=m_v, in0=h_v, in1=ee_v,
                                     op=mybir.AluOpType.mult)

                for bi in range(nbc):
                    b = ci.b0 + bi
                    msg = sb.tile([P, F1], F32, tag="msg")
                    den = sb.tile([P, h1], F32, tag="den")
                    mw = sb.tile([P, F1], F32, tag="msgw")
                    dw = sb.tile([P, h1], F32, tag="denw")
                    for wv in range(NW):
                        L = int(Lbw[b, wv])
                        o = int(ci.wstart[wv] + Lbw[ci.b0:b, wv].sum())
                        mt = msg if wv == 0 else mw
                        dt = den if wv == 0 else dw
                        nc.vector.tensor_reduce(
                            out=mt[:, 0:F], in_=m[:, 0:F, o:o + L],
                            axis=mybir.AxisListType.X,
                            op=mybir.AluOpType.add)
                        nc.vector.tensor_reduce(
                            out=dt[:, 0:H],
                            in_=ee[:, o:o + L, 0:H]
                            .rearrange("p l h -> p h l"),
                            axis=mybir.AxisListType.X,
                            op=mybir.AluOpType.add)
                        if wv > 0:
                            nc.vector.tensor_tensor(
                                out=msg[:, 0:F], in0=msg[:, 0:F],
                                in1=mw[:, 0:F], op=mybir.AluOpType.add)
                            nc.vector.tensor_tensor(
                                out=den[:, 0:H], in0=den[:, 0:H],
                                in1=dw[:, 0:H], op=mybir.AluOpType.add)

                    rec = sb.tile([P, h1], F32, tag="rec")
                    nc.vector.reciprocal(rec[:, 0:H], den[:, 0:H])
                    o1 = sb.tile([P, F1], F32, tag="o1")
                    nc.vector.tensor_tensor(
                        out=o1[:, 0:F].rearrange("p (h c) -> p h c", h=H),
                        in0=msg[:, 0:F].rearrange("p (h c) -> p h c", h=H),
                        in1=rec[:, 0:H].unsqueeze(2)
                        .broadcast_to([P, H, C]),
                        op=mybir.AluOpType.mult)
                    nc.vector.tensor_tensor(out=o1[:, 0:F], in0=o1[:, 0:F],
                                            in1=bias[:, 0:F],
                                            op=mybir.AluOpType.add)

                    if layer == 1:
                        t1_ = sb.tile([P, F1], F32, tag="elu1")
                        nc.scalar.activation(t1_[:], o1[:],
                                             mybir.ActivationFunctionType.Exp)
                        nc.vector.tensor_scalar_min(t1_[:], t1_[:], 1.0)
                        t2_ = sb.tile([P, F1], F32, tag="elu2")
                        nc.scalar.activation(
                            t2_[:], o1[:],
                            mybir.ActivationFunctionType.Relu)
                        nc.vector.tensor_tensor(out=t1_[:], in0=t1_[:],
                                                in1=t2_[:],
                                                op=mybir.AluOpType.add)
                        h2 = sb.tile([P, F1], BF16, tag="h2")
                        nc.vector.tensor_scalar_add(h2[:], t1_[:], -1.0)

                        pst = ps.tile([P, P], BF16, tag="ps_t")
                        nc.tensor.transpose(pst[:], h2[:], ident[:])
                        h2T = sb.tile([P, P], BF16, tag="h2T")
                        nc.vector.tensor_copy(h2T[:], pst[:])
                        p2 = ps.tile([P, ncls + 2], F32, tag="ps_2")
                        nc.tensor.matmul(p2[:], lhsT=h2T[:], rhs=W2aug[:],
                                         start=True, stop=True)
                        if stctr[0] is None or stctr[1] == PG:
                            stctr[0] = ph1.tile([P, PG, ncls + 1], BF16,
                                                tag="t2stage",
                                                name="t2stage")
                            stctr[1] = 0
                            stctr[2] = b
                        st, fi = stctr[0], stctr[1]
                        nc.vector.tensor_copy(st[:, fi, 0:ncls + 1],
                                              p2[:, 0:ncls + 1])
                        nc.vector.tensor_copy(adn2sb[:, b:b + 1],
                                              p2[:, ncls + 1:ncls + 2])
                        stctr[1] += 1
                        if stctr[1] == PG:
                            sb0 = stctr[2]
                            nc.scalar.dma_start(
                                t2s[sb0 * P:(sb0 + PG) * P, :].rearrange(
                                    "(b p) r -> p b r", p=P),
                                st[:])
                    else:
                        ex = sb.tile([P, ncls], F32, tag="lsm_e")
                        ssum = sb.tile([P, 1], F32, tag="lsm_s")
                        nc.scalar.activation(ex[:], o1[:, 0:F],
                                             mybir.ActivationFunctionType.Exp,
                                             accum_out=ssum[:])
                        ln = sb.tile([P, 1], F32, tag="lsm_l")
                        nc.scalar.activation(ln[:], ssum[:],
                                             mybir.ActivationFunctionType.Ln)
                        if stctr[0] is None or stctr[1] == PG:
                            stctr[0] = ph1.tile([P, PG, ncls], F32,
                                                tag="ostage",
                                                name="ostage")
                            stctr[1] = 0
                            stctr[2] = b
                        st, fi = stctr[0], stctr[1]
                        nc.vector.tensor_tensor(
                            out=st[:, fi, :], in0=o1[:, 0:F],
                            in1=ln[:].broadcast_to([P, F]),
                            op=mybir.AluOpType.subtract)
                        stctr[1] += 1
                        if stctr[1] == PG:
                            sb0 = stctr[2]
                            nc.scalar.dma_start(
                                outD[sb0 * P:(sb0 + PG) * P, :].rearrange(
                                    "(b p) r -> p b r", p=P),
                                st[:])

                if layer == 1:
                    for k in range(NW):
                        if ci.b1 == (k + 1) * NBQ and do_compute \
                                and "noag2" not in BUILD_VARIANT:
                            nc.sync.dma_start(
                                t2s[k * WINB + SENT:
                                    k * WINB + SENT + 1, :], s2[:])
                            nc.gpsimd.collective_compute(
                                "AllGather", mybir.AluOpType.bypass,
                                replica_groups=rg,
                                ins=[t2s[k * WINB:(k + 1) * WINB, :].opt()],
                                outs=[t2p[k * WIN:(k + 1) * WIN, :].opt()])
                            nc.sync.dma_start(
                                t2f[k * WIN:(k + 1) * WIN, 0:R2P],
                                t2p[k * WIN:(k + 1) * WIN, :])

        if BUILD_VARIANT != "p1":
            edge_phase(1)
        if BUILD_VARIANT == "full":
            edge_phase(2)

    nc.compile()

    if QUEUE_RR:
        # Tile assigned DMASW lanes in scheduled order; pair each gather's
        # SWDGE queue with its lane (lane % 4) so a semaphore lane only
        # ever serves one queue (HW shadow-sem bookkeeping requirement).
        for f in nc.m.functions:
            for blk in f.blocks:
                for ins in blk.instructions:
                    if isinstance(ins, mybir.InstDMAGatherAnt):
                        si = ins.sync_info
                        lane = None
                        if si is not None:
                            for u in si.on_update:
                                nm = u.ant_name or ""
                                if nm.startswith("DMASW"):
                                    lane = int(nm[5:].split("_")[0])
                        if lane is not None:
                            ins.queue_num = lane % 4
    return nc


# ---------------------------------------------------------------------------
# Entry point
# ---------------------------------------------------------------------------

def _block_diag_a(a_src, a_dst):
    h, c = a_src.shape
    F1 = h * c
    ab = np.zeros((F1, 2 * h), dtype=np.float32)
    for hd in range(h):
        ab[hd * c:(hd + 1) * c, hd] = a_src[hd]
        ab[hd * c:(hd + 1) * c, h + hd] = a_dst[hd]
    return ab


def prepare(x, edge_index, W1, a_src1, a_dst1, b1, W2, a_src2, a_dst2, b2):
    import ml_dtypes
    x = np.asarray(x, dtype=np.float32)
    edge_index = np.asarray(edge_index)
    n_nodes, f_in = x.shape
    h1, c1 = np.asarray(a_src1).shape
    ncls = np.asarray(W2).shape[1]

    plan = build_plan(edge_index, n_nodes)
    nc = build_program(plan, f_in, h1, c1, ncls)

    AB1 = _block_diag_a(np.asarray(a_src1, np.float32),
                        np.asarray(a_dst1, np.float32))
    W1f = np.asarray(W1, np.float32)
    W1a = np.concatenate([W1f, W1f @ AB1], axis=1).astype(ml_dtypes.bfloat16)
    A2 = np.concatenate([np.asarray(a_src2, np.float32).T,
                         np.asarray(a_dst2, np.float32).T], axis=1)
    common = {
        "W1a": W1a,
        "b1r": np.asarray(b1, np.float32).reshape(1, -1),
        "W2": np.ascontiguousarray(W2, np.float32),
        "W2T": np.ascontiguousarray(np.asarray(W2, np.float32).T),
        "A2": np.ascontiguousarray(A2),
        "b2r": np.asarray(b2, np.float32).reshape(1, -1),
    }
    in_maps = []
    for c in range(N_CORES):
        pl = plan.cores[c]
        im = dict(common)
        xs = np.zeros((plan.pos, f_in), dtype=np.float32)
        xs[pl.posq] = x[c * plan.shard:(c + 1) * plan.shard]
        im["xT"] = np.ascontiguousarray(xs.T).astype(ml_dtypes.bfloat16)
        im["gidx"] = pl.gidx
        in_maps.append(im)
    return plan, nc, in_maps, (n_nodes, ncls)


def finish(plan, shard_outs, n_nodes, ncls):
    out = np.empty((n_nodes, ncls), dtype=np.float32)
    for c in range(N_CORES):
        pl = plan.cores[c]
        out[c * plan.shard:(c + 1) * plan.shard] = shard_outs[c][pl.posq]
    return out


def kernel(x, edge_index, W1, a_src1, a_dst1, b1, W2, a_src2, a_dst2, b2,
           **run_kwargs):
    plan, nc, in_maps, (n_nodes, ncls) = prepare(
        x, edge_index, W1, a_src1, a_dst1, b1, W2, a_src2, a_dst2, b2)
    res = run_bass_kernel_spmd(nc, in_maps, core_ids=list(range(N_CORES)),
                               **run_kwargs)
    out = finish(plan, [res.results[c]["out"] for c in range(N_CORES)],
                 n_nodes, ncls)
    kernel.last_result = res
    return out


# revision 4
# speedup vs baseline: 1.0058x; 1.0058x over previous
"""Two-layer GAT (PyG GATConv semantics) on 8 Trainium2 NeuronCores — v3.

Gather strategy: batched SWDGE dma_gather (~0.34ns/row descriptor cost)
instead of v1's per-slot indirect DMAs (~1us fixed cost each, ~4000 of
them = 4ms).

dma_gather constraints shape the whole layout:
  - int16 row indices -> the node table is cut into NW=4 windows; each
    gather call addresses one window (idx relative to window base).
  - Rows must be a multiple of 256B -> layer-1 rows pack h as fp8 (128B)
    + asn as bf16 (16B); layer-2 rows pack h2 bf16 (128B) + asn2 (2B).
  - Row i of a call lands at SBUF (partition i%128, slot i//128), and all
    indices must be valid -> padding slots point at a per-window sentinel
    row (h=0, asn=-60 so exp(lrelu(...)) ~ 0).

Node placement (host side): a greedy + refined 4-coloring balances each
dst's in-edge sources across the 4 windows (cuts slot padding from ~2.0x
to ~1.6x); within each (core, window) block nodes are in-degree sorted
into batches of 128 so the per-batch max degree (column count) is small.

Per chunk of batches, slab columns are window-major: [w0 blocks | w1 |
w2 | w3], one dma_gather per (chunk, window). Softmax denominators and
messages are per-batch reductions assembled from 4 window partials.

AllGather chunks are window-aligned and interleaved with compute so
edge-phase gathers start as soon as their window's table section lands.
"""

import sys

for _p in ("/opt/trn_rl_repo",):
    if _p not in sys.path:
        sys.path.insert(0, _p)

import numpy as np

from concourse import bacc, bass, mybir, tile
from concourse.bass_utils import run_bass_kernel_spmd
from concourse.masks import make_identity

F32 = mybir.dt.float32
BF16 = mybir.dt.bfloat16
F8 = mybir.dt.float8e4
I16 = mybir.dt.int16

N_CORES = 8
P = 128
NW = 4
NEG_SLOPE = 0.2
SENT_ASN = -60.0
ROWB = 256          # table row bytes
RE = ROWB // 2      # row length in bf16 elements
S_CHUNK = 176       # max slab columns per chunk
BUILD_VARIANT = "full"   # "p1" | "e1" | "full" (phase bisection)
QUEUE_RR = True          # round-robin dma_gather queue_num across 0..3
SIM_ZERO_PAD = False     # zero table pad columns (sim NaN checker only)


class Plan:
    pass


# ---------------------------------------------------------------------------
# Host-side graph planning
# ---------------------------------------------------------------------------

def _color_windows(src, dst, n_nodes, shard, winb):
    order = np.argsort(src, kind="stable")
    d_sorted = dst[order]
    starts = np.searchsorted(src[order], np.arange(n_nodes + 1))

    cnt = np.zeros((n_nodes, NW), dtype=np.int32)
    cap = np.zeros((N_CORES, NW), dtype=np.int64)
    capmax = winb - 1
    color = np.zeros(n_nodes, dtype=np.int8)

    rng = np.random.default_rng(0)
    B = 128

    def _sweep(first):
        for i in range(0, n_nodes, B):
            us = proc[i:i + B]
            counts = starts[us + 1] - starts[us]
            rows = np.repeat(np.arange(len(us)), counts)
            flat = np.concatenate(
                [d_sorted[starts[u]:starts[u + 1]] for u in us])
            score = np.zeros((len(us), NW), dtype=np.float64)
            np.add.at(score, rows, cnt[flat])
            cores = us // shard
            w0 = color[us]
            if not first:
                score[np.arange(len(us)), w0] -= counts
            full = cap[cores] >= capmax
            if not first:
                full[np.arange(len(us)), w0] = False
            score[full] = np.inf
            w = np.argmin(score, axis=1).astype(np.int8)
            if first:
                color[us] = w
                np.add.at(cap, (cores, w), 1)
                np.add.at(cnt, (flat, w[rows]), 1)
            else:
                moved = w != w0
                um = us[moved]
                color[um] = w[moved]
                np.add.at(cap, (um // shard, w[moved]), 1)
                np.add.at(cap, (um // shard, w0[moved]), -1)
                mrows = moved[rows]
                np.add.at(cnt, (flat[mrows], w[rows][mrows]), 1)
                np.add.at(cnt, (flat[mrows], w0[rows][mrows]), -1)

    proc = rng.permutation(n_nodes)
    _sweep(True)
    # capacity repair: block-greedy can overshoot capmax slightly
    for c in range(N_CORES):
        ids = np.arange(c * shard, (c + 1) * shard)
        for wv in range(NW):
            over = int(cap[c, wv]) - capmax
            if over <= 0:
                continue
            sel = ids[color[ids] == wv][:over]
            for u in sel:
                w2 = int(np.argmin(cap[c]))
                ds = d_sorted[starts[u]:starts[u + 1]]
                cnt[ds, wv] -= 1
                cnt[ds, w2] += 1
                cap[c, wv] -= 1
                cap[c, w2] += 1
                color[u] = w2
    for _ in range(4):
        proc = rng.permutation(n_nodes)
        _sweep(False)
    for c in range(N_CORES):
        assert cap[c].max() <= capmax
    return color, cnt


def build_plan(edge_index, n_nodes):
    assert n_nodes % N_CORES == 0
    shard = n_nodes // N_CORES
    winb = -(-shard // (NW * P)) * P       # rows per (core, window) block
    pos = NW * winb                        # padded positions per core
    nbq = winb // P                        # batches per quarter
    nb = NW * nbq
    win = N_CORES * winb                   # rows per window in full table
    sent_local = winb - 1
    pg = 5 if nbq % 5 == 0 else 1          # staging group (divides nbq)

    src_all = np.concatenate(
        [edge_index[0].astype(np.int64), np.arange(n_nodes, dtype=np.int64)])
    dst_all = np.concatenate(
        [edge_index[1].astype(np.int64), np.arange(n_nodes, dtype=np.int64)])

    color, cnt = _color_windows(src_all, dst_all, n_nodes, shard, winb)
    owner = dst_all // shard

    posq_cores, deg_cores, edges_cores = [], [], []
    for c in range(N_CORES):
        m = owner == c
        es = src_all[m]
        ed = dst_all[m] - c * shard
        o = np.argsort(ed, kind="stable")
        es, ed = es[o], ed[o]
        deg = np.bincount(ed, minlength=shard)
        assert deg.min() >= 1
        posq = np.empty(shard, dtype=np.int64)
        col_local = color[c * shard:(c + 1) * shard]
        for k in range(NW):
            sel = np.where(col_local == k)[0]
            assert len(sel) <= winb - 1
            o2 = np.argsort(-deg[sel], kind="stable")
            posq[sel[o2]] = k * winb + np.arange(len(sel))
        posq_cores.append(posq)
        deg_cores.append(deg)
        edges_cores.append((es, ed))

    posmap = np.empty(n_nodes, dtype=np.int64)
    for c in range(N_CORES):
        q = posq_cores[c]
        k = q // winb
        posmap[c * shard:(c + 1) * shard] = \
            k * win + c * winb + (q - k * winb)

    # shared per-(batch, window) column counts (max over cores+partitions)
    Lbw = np.zeros((nb, NW), dtype=np.int64)
    for c in range(N_CORES):
        cw = np.zeros((pos, NW), dtype=np.int64)
        cw[posq_cores[c]] = cnt[c * shard:(c + 1) * shard]
        Lbw = np.maximum(Lbw, cw.reshape(nb, P, NW).max(axis=1))
    Lbw = np.maximum(Lbw, 1)

    # chunks of batches (never crossing quarter boundaries)
    Lb = Lbw.sum(axis=1)
    chunks = []
    b = 0
    while b < nb:
        e = b
        s = 0
        while (e < nb and e // nbq == b // nbq and s + Lb[e] <= S_CHUNK):
            s += Lb[e]
            e += 1
        if e == b:
            e = b + 1
        chunks.append((b, e))
        b = e

    chunk_info = []
    gidx_cols = 0
    boff_full = np.zeros((nb, NW), dtype=np.int64)   # col of (b, w) block
    gcol_of_b = np.zeros(nb, dtype=np.int64)
    for (b0, b1) in chunks:
        Sw = Lbw[b0:b1].sum(axis=0)
        S = int(Sw.sum())
        wstart = np.concatenate([[0], np.cumsum(Sw)])[:NW]
        for wv in range(NW):
            boff_full[b0:b1, wv] = wstart[wv] + np.concatenate(
                [[0], np.cumsum(Lbw[b0:b1, wv])])[:b1 - b0]
        gcol_of_b[b0:b1] = gidx_cols
        ci = Plan()
        ci.b0, ci.b1, ci.S = b0, b1, S
        ci.Sw = Sw.astype(int)
        ci.wstart = wstart.astype(int)
        ci.gcol = gidx_cols
        gidx_cols += 8 * S
        chunk_info.append(ci)

    pc = []
    for c in range(N_CORES):
        es, ed = edges_cores[c]
        posq = posq_cores[c]

        erow = posmap[es]
        ew = erow // win
        eidx = erow - ew * win
        q = posq[ed]
        eb = q // P
        ep = q % P

        # rank within (dst, window) run
        o3 = np.lexsort((ew, ed))
        ed2, ew2, eidx2, eb2, ep2 = \
            ed[o3], ew[o3], eidx[o3], eb[o3], ep[o3]
        key = ed2 * NW + ew2
        first_of = np.r_[True, key[1:] != key[:-1]]
        run_id = np.cumsum(first_of) - 1
        run_start = np.where(first_of)[0]
        j_in_run = np.arange(len(ed2)) - run_start[run_id]

        col_e = boff_full[eb2, ew2] + j_in_run
        i_e = col_e * P + ep2
        gc = gcol_of_b[eb2] + i_e // 16

        gidx = np.full((16, gidx_cols), sent_local, dtype=np.int16)
        gidx[(i_e % 16).astype(np.int64), gc.astype(np.int64)] = \
            eidx2.astype(np.int16)

        pl = Plan()
        pl.gidx = np.tile(gidx, (8, 1))
        pl.posq = posq
        pc.append(pl)

    plan = Plan()
    plan.shard, plan.winb, plan.pos, plan.nbq, plan.nb = \
        shard, winb, pos, nbq, nb
    plan.win, plan.sent_local, plan.pg = win, sent_local, pg
    plan.Lbw, plan.chunks, plan.chunk_info = Lbw, chunks, chunk_info
    plan.gidx_cols = gidx_cols
    plan.cores = pc
    return plan


# ---------------------------------------------------------------------------
# Device program
# ---------------------------------------------------------------------------

def build_program(plan, f_in, h1, c1, ncls):
    F1 = h1 * c1
    GC = plan.gidx_cols
    POS, WINB, WIN = plan.pos, plan.winb, plan.win
    NB, NBQ, PG = plan.nb, plan.nbq, plan.pg
    SENT = plan.sent_local
    Lbw, chunk_info = plan.Lbw, plan.chunk_info

    nc = bacc.Bacc(target_bir_lowering=False, debug=False,
                   num_devices=N_CORES, num_swdge_queues=4,
                   dynamic_dma_scratch_size=32768)

    xT = nc.declare_dram_parameter("xT", [f_in, POS], BF16, isOutput=False)
    W1a = nc.declare_dram_parameter("W1a", [f_in, F1 + 2 * h1], BF16,
                                    isOutput=False)
    W2 = nc.declare_dram_parameter("W2", [F1, ncls], F32, isOutput=False)
    W2T = nc.declare_dram_parameter("W2T", [ncls, F1], F32, isOutput=False)
    A2 = nc.declare_dram_parameter("A2", [ncls, 2], F32, isOutput=False)
    b1r = nc.declare_dram_parameter("b1r", [1, F1], F32, isOutput=False)
    b2r = nc.declare_dram_parameter("b2r", [1, ncls], F32, isOutput=False)
    gidxD = nc.declare_dram_parameter("gidx", [P, GC], I16, isOutput=False)
    outD = nc.declare_dram_parameter("out", [POS, ncls], F32, isOutput=True)

    R1P = 64 + h1                 # packed t1 row elements (bf16)
    R2P = ncls + 1                # packed t2 row elements
    t1s = nc.dram_tensor("t1shard", [POS, R1P], BF16)
    t1p = nc.dram_tensor("t1pack", [N_CORES * POS, R1P], BF16,
                         addr_space="Shared")
    t1f = nc.dram_tensor("t1full", [N_CORES * POS, RE], BF16)
    t2s = nc.dram_tensor("t2shard", [POS, R2P], BF16)
    t2p = nc.dram_tensor("t2pack", [N_CORES * POS, R2P], BF16,
                         addr_space="Shared")
    t2f = nc.dram_tensor("t2full", [N_CORES * POS, RE], BF16)

    rg = [list(range(N_CORES))]

    from contextlib import ExitStack
    with tile.TileContext(nc) as tc, ExitStack() as ctx:
        const = ctx.enter_context(tc.tile_pool(name="const", bufs=1))
        sb = ctx.enter_context(tc.tile_pool(name="sb", bufs=2))
        ph1 = ctx.enter_context(tc.tile_pool(name="ph1", bufs=2))
        slabp = ctx.enter_context(tc.tile_pool(name="slabp", bufs=2))
        mp = ctx.enter_context(tc.tile_pool(name="mp", bufs=1))
        ixp = ctx.enter_context(tc.tile_pool(name="ixp", bufs=4))
        psc = ctx.enter_context(tc.tile_pool(name="psc", bufs=1,
                                             space="PSUM"))
        ps = ctx.enter_context(tc.tile_pool(name="ps", bufs=2, space="PSUM"))

        # ------------------- constants -------------------
        W1aug = const.tile([f_in, F1 + 2 * h1], BF16)
        nc.sync.dma_start(W1aug[:], W1a[:, :])
        W2sb = const.tile([F1, ncls], F32)
        nc.sync.dma_start(W2sb[:], W2[:, :])
        W2Tsb = const.tile([ncls, F1], F32)
        nc.sync.dma_start(W2Tsb[:], W2T[:, :])
        A2sb = const.tile([ncls, 2], F32)
        nc.sync.dma_start(A2sb[:], A2[:, :])
        b1row = const.tile([1, F1], F32)
        nc.sync.dma_start(b1row[:], b1r[:, :])
        b2row = const.tile([1, ncls], F32)
        nc.sync.dma_start(b2row[:], b2r[:, :])

        psB = psc.tile([F1, 2], F32, tag="ps_small")
        nc.tensor.matmul(psB[:], lhsT=W2Tsb[:], rhs=A2sb[:], start=True,
                         stop=True)
        W2aug = const.tile([F1, ncls + 2], BF16)
        nc.vector.tensor_copy(W2aug[:, 0:ncls], W2sb[:])
        nc.vector.tensor_copy(W2aug[:, ncls:ncls + 2], psB[:])

        ones1 = const.tile([1, P], F32)
        nc.vector.memset(ones1[:], 1.0)
        psb1 = psc.tile([P, F1], F32, tag="ps_bias")
        nc.tensor.matmul(psb1[:], lhsT=ones1[:], rhs=b1row[:], start=True,
                         stop=True)
        b1bc = const.tile([P, F1], F32)
        nc.vector.tensor_copy(b1bc[:], psb1[:])
        psb2 = psc.tile([P, ncls], F32, tag="ps_bias")
        nc.tensor.matmul(psb2[:], lhsT=ones1[:], rhs=b2row[:], start=True,
                         stop=True)
        b2bc = const.tile([P, ncls], F32)
        nc.vector.tensor_copy(b2bc[:], psb2[:])

        ident = const.tile([P, P], BF16)
        make_identity(nc, ident[:])

        # sentinel rows (element layout: [0:64]=h (fp8 bytes), 64..=asn)
        s1 = const.tile([1, 64 + h1], BF16)
        nc.vector.memset(s1[:], 0.0)
        nc.vector.memset(s1[:, 64:64 + h1], SENT_ASN)
        s2 = const.tile([1, ncls + 1], BF16)
        nc.vector.memset(s2[:], 0.0)
        nc.vector.memset(s2[:, ncls:ncls + 1], SENT_ASN)

        adn1sb = const.tile([P, NB, h1], BF16)
        adn2sb = const.tile([P, NB], BF16)

        if SIM_ZERO_PAD:
            zpad = const.tile([P, RE], BF16)
            nc.vector.memset(zpad[:], 0.0)
            ngr = N_CORES * POS // P
            for gi in range(ngr):
                nc.sync.dma_start(
                    t1f[gi * P:(gi + 1) * P, 64 + h1:RE],
                    zpad[:, 0:RE - 64 - h1])
                nc.sync.dma_start(
                    t2f[gi * P:(gi + 1) * P, ncls + 1:RE],
                    zpad[:, 0:RE - ncls - 1])

        # ---- phase 1 ----
        for g in range(NB // PG):
            b0 = g * PG
            xt = ph1.tile([f_in, PG * P], BF16, tag="xt")
            nc.sync.dma_start(xt[:], xT[:, b0 * P:(b0 + PG) * P])
            stage = ph1.tile([P, PG, 64 + h1], BF16, tag="stage")
            for bi in range(PG):
                b = b0 + bi
                p1 = ps.tile([P, F1 + 2 * h1], F32, tag="ps_p1")
                nc.tensor.matmul(p1[:], lhsT=xt[:, bi * P:(bi + 1) * P],
                                 rhs=W1aug[:], start=True, stop=True)
                nc.vector.tensor_copy(stage[:, bi, 0:F1 // 2].bitcast(F8),
                                      p1[:, 0:F1])
                nc.vector.tensor_copy(stage[:, bi, 64:64 + h1],
                                      p1[:, F1:F1 + h1])
                nc.vector.tensor_copy(adn1sb[:, b, :],
                                      p1[:, F1 + h1:F1 + 2 * h1])
            nc.scalar.dma_start(
                t1s[b0 * P:(b0 + PG) * P, :].rearrange(
                    "(b p) r -> p b r", p=P),
                stage[:])
            if (b0 + PG) % NBQ == 0 and "noag1" not in BUILD_VARIANT:
                k = (b0 + PG) // NBQ - 1
                nc.sync.dma_start(
                    t1s[k * WINB + SENT:k * WINB + SENT + 1, :], s1[:])
                nc.gpsimd.collective_compute(
                    "AllGather", mybir.AluOpType.bypass, replica_groups=rg,
                    ins=[t1s[k * WINB:(k + 1) * WINB, :].opt()],
                    outs=[t1p[k * WIN:(k + 1) * WIN, :].opt()])
                nc.sync.dma_start(t1f[k * WIN:(k + 1) * WIN, 0:R1P],
                                  t1p[k * WIN:(k + 1) * WIN, :])

        # ------------------- edge phases -------------------
        def edge_phase(layer):
            do_gather = "nogather" not in BUILD_VARIANT
            do_compute = "nocompute" not in BUILD_VARIANT
            table = t1f if layer == 1 else t2f
            H = h1 if layer == 1 else 1
            F = F1 if layer == 1 else ncls
            C = F // H
            AOFF = 64 if layer == 1 else ncls    # asn element offset in row
            bias = b1bc if layer == 1 else b2bc

            stctr = [None, 0, 0]  # tile, fill, first batch

            for ci in chunk_info:
                S = ci.S
                nbc = ci.b1 - ci.b0

                slab = slabp.tile([P, S_CHUNK, RE], BF16, tag="slab")
                for wv in range(NW):
                    Sw = int(ci.Sw[wv])
                    if Sw == 0 or not do_gather:
                        continue
                    ix = ixp.tile([P, 8 * S_CHUNK], I16, tag="ix")
                    g0 = ci.gcol + 8 * int(ci.wstart[wv])
                    nc.sync.dma_start(ix[:, 0:8 * Sw],
                                      gidxD[:, g0:g0 + 8 * Sw])
                    nc.gpsimd.dma_gather(
                        out_ap=slab[:,
                                    int(ci.wstart[wv]):
                                    int(ci.wstart[wv]) + Sw, :],
                        in_ap=table[wv * WIN:(wv + 1) * WIN, :],
                        idxs_ap=ix[:, 0:8 * Sw],
                        num_idxs=P * Sw, num_idxs_reg=P * Sw,
                        elem_size=RE, single_packet=False)

                if not do_compute:
                    continue
                adnx = sb.tile([P, S_CHUNK, h1], BF16, tag="adnx")
                for bi in range(nbc):
                    b = ci.b0 + bi
                    for wv in range(NW):
                        L = int(Lbw[b, wv])
                        o = int(ci.wstart[wv] + Lbw[ci.b0:b, wv].sum())
                        if layer == 1:
                            srcv = adn1sb[:, b:b + 1, :] \
                                .broadcast_to([P, L, H])
                        else:
                            srcv = adn2sb[:, b:b + 1].unsqueeze(2) \
                                .broadcast_to([P, L, H])
                        nc.vector.tensor_copy(adnx[:, o:o + L, 0:H], srcv)

                e = sb.tile([P, S_CHUNK, h1], F32, tag="e")
                ev = e[:, 0:S, 0:H]
                nc.vector.tensor_tensor(
                    out=ev, in0=slab[:, 0:S, AOFF:AOFF + H],
                    in1=adnx[:, 0:S, 0:H], op=mybir.AluOpType.add)
                nc.vector.scalar_tensor_tensor(
                    out=ev, in0=ev, scalar=NEG_SLOPE, in1=ev,
                    op0=mybir.AluOpType.mult, op1=mybir.AluOpType.max)
                ee = sb.tile([P, S_CHUNK, h1], BF16, tag="ee")
                nc.scalar.activation(ee[:, 0:S, 0:H], ev,
                                     mybir.ActivationFunctionType.Exp)

                m = mp.tile([P, F1, S_CHUNK], BF16, tag="m")
                if layer == 1:
                    m_v = m[:, :, 0:S].rearrange("p (h c) l -> p h c l",
                                                 h=H)
                    h_v = slab[:, 0:S, 0:F1 // 2].bitcast(F8) \
                        .rearrange("p l (h c) -> p h c l", h=H)
                    ee_v = ee[:, 0:S, 0:H].rearrange("p l h -> p h l") \
                        .unsqueeze(2).broadcast_to([P, H, C, S])
                else:
                    m_v = m[:, 0:F, 0:S]
                    h_v = slab[:, 0:S, 0:ncls].rearrange("p l c -> p c l")
                    ee_v = ee[:, 0:S, 0:1].rearrange("p l h -> p h l") \
                        .broadcast_to([P, C, S])
                nc.any.tensor_tensor(out=m_v, in0=h_v, in1=ee_v,
                                     op=mybir.AluOpType.mult)

                for bi in range(nbc):
                    b = ci.b0 + bi
                    msg = sb.tile([P, F1], F32, tag="msg")
                    den = sb.tile([P, h1], F32, tag="den")
                    mw = sb.tile([P, F1], F32, tag="msgw")
                    dw = sb.tile([P, h1], F32, tag="denw")
                    for wv in range(NW):
                        L = int(Lbw[b, wv])
                        o = int(ci.wstart[wv] + Lbw[ci.b0:b, wv].sum())
                        mt = msg if wv == 0 else mw
                        dt = den if wv == 0 else dw
                        nc.vector.tensor_reduce(
                            out=mt[:, 0:F], in_=m[:, 0:F, o:o + L],
                            axis=mybir.AxisListType.X,
                            op=mybir.AluOpType.add)
                        nc.vector.tensor_reduce(
                            out=dt[:, 0:H],
                            in_=ee[:, o:o + L, 0:H]
                            .rearrange("p l h -> p h l"),
                            axis=mybir.AxisListType.X,
                            op=mybir.AluOpType.add)
                        if wv > 0:
                            nc.vector.tensor_tensor(
                                out=msg[:, 0:F], in0=msg[:, 0:F],
                                in1=mw[:, 0:F], op=mybir.AluOpType.add)
                            nc.vector.tensor_tensor(
                                out=den[:, 0:H], in0=den[:, 0:H],
                                in1=dw[:, 0:H], op=mybir.AluOpType.add)

                    rec = sb.tile([P, h1], F32, tag="rec")
                    nc.vector.reciprocal(rec[:, 0:H], den[:, 0:H])
                    o1 = sb.tile([P, F1], F32, tag="o1")
                    nc.vector.tensor_tensor(
                        out=o1[:, 0:F].rearrange("p (h c) -> p h c", h=H),
                        in0=msg[:, 0:F].rearrange("p (h c) -> p h c", h=H),
                        in1=rec[:, 0:H].unsqueeze(2)
                        .broadcast_to([P, H, C]),
                        op=mybir.AluOpType.mult)
                    nc.vector.tensor_tensor(out=o1[:, 0:F], in0=o1[:, 0:F],
                                            in1=bias[:, 0:F],
                                            op=mybir.AluOpType.add)

                    if layer == 1:
                        t1_ = sb.tile([P, F1], F32, tag="elu1")
                        nc.scalar.activation(t1_[:], o1[:],
                                             mybir.ActivationFunctionType.Exp)
                        nc.vector.tensor_scalar_min(t1_[:], t1_[:], 1.0)
                        t2_ = sb.tile([P, F1], F32, tag="elu2")
                        nc.scalar.activation(
                            t2_[:], o1[:],
                            mybir.ActivationFunctionType.Relu)
                        nc.vector.tensor_tensor(out=t1_[:], in0=t1_[:],
                                                in1=t2_[:],
                                                op=mybir.AluOpType.add)
                        h2 = sb.tile([P, F1], BF16, tag="h2")
                        nc.vector.tensor_scalar_add(h2[:], t1_[:], -1.0)

                        pst = ps.tile([P, P], BF16, tag="ps_t")
                        nc.tensor.transpose(pst[:], h2[:], ident[:])
                        h2T = sb.tile([P, P], BF16, tag="h2T")
                        nc.vector.tensor_copy(h2T[:], pst[:])
                        p2 = ps.tile([P, ncls + 2], F32, tag="ps_2")
                        nc.tensor.matmul(p2[:], lhsT=h2T[:], rhs=W2aug[:],
                                         start=True, stop=True)
                        if stctr[0] is None or stctr[1] == PG:
                            stctr[0] = ph1.tile([P, PG, ncls + 1], BF16,
                                                tag="t2stage",
                                                name="t2stage")
                            stctr[1] = 0
                            stctr[2] = b
                        st, fi = stctr[0], stctr[1]
                        nc.vector.tensor_copy(st[:, fi, 0:ncls + 1],
                                              p2[:, 0:ncls + 1])
                        nc.vector.tensor_copy(adn2sb[:, b:b + 1],
                                              p2[:, ncls + 1:ncls + 2])
                        stctr[1] += 1
                        if stctr[1] == PG:
                            sb0 = stctr[2]
                            nc.scalar.dma_start(
                                t2s[sb0 * P:(sb0 + PG) * P, :].rearrange(
                                    "(b p) r -> p b r", p=P),
                                st[:])
                    else:
                        ex = sb.tile([P, ncls], F32, tag="lsm_e")
                        ssum = sb.tile([P, 1], F32, tag="lsm_s")
                        nc.scalar.activation(ex[:], o1[:, 0:F],
                                             mybir.ActivationFunctionType.Exp,
                                             accum_out=ssum[:])
                        ln = sb.tile([P, 1], F32, tag="lsm_l")
                        nc.scalar.activation(ln[:], ssum[:],
                                             mybir.ActivationFunctionType.Ln)
                        if stctr[0] is None or stctr[1] == PG:
                            stctr[0] = ph1.tile([P, PG, ncls], F32,
                                                tag="ostage",
                                                name="ostage")
                            stctr[1] = 0
                            stctr[2] = b
                        st, fi = stctr[0], stctr[1]
                        nc.vector.tensor_tensor(
                            out=st[:, fi, :], in0=o1[:, 0:F],
                            in1=ln[:].broadcast_to([P, F]),
                            op=mybir.AluOpType.subtract)
                        stctr[1] += 1
                        if stctr[1] == PG:
                            sb0 = stctr[2]
                            nc.scalar.dma_start(
                                outD[sb0 * P:(sb0 + PG) * P, :].rearrange(
                                    "(b p) r -> p b r", p=P),
                                st[:])

                if layer == 1:
                    for k in range(NW):
                        fire = (ci.b1 == (k + 2) * NBQ) or \
                            (k == NW - 1 and ci.b1 == NW * NBQ)
                        if fire and do_compute \
                                and "noag2" not in BUILD_VARIANT:
                            nc.sync.dma_start(
                                t2s[k * WINB + SENT:
                                    k * WINB + SENT + 1, :], s2[:])
                            nc.gpsimd.collective_compute(
                                "AllGather", mybir.AluOpType.bypass,
                                replica_groups=rg,
                                ins=[t2s[k * WINB:(k + 1) * WINB, :].opt()],
                                outs=[t2p[k * WIN:(k + 1) * WIN, :].opt()])
                            nc.sync.dma_start(
                                t2f[k * WIN:(k + 1) * WIN, 0:R2P],
                                t2p[k * WIN:(k + 1) * WIN, :])

        if BUILD_VARIANT != "p1":
            edge_phase(1)
        if BUILD_VARIANT == "full":
            edge_phase(2)

    nc.compile()

    if QUEUE_RR:
        # Tile assigned DMASW lanes in scheduled order; pair each gather's
        # SWDGE queue with its lane (lane % 4) so a semaphore lane only
        # ever serves one queue (HW shadow-sem bookkeeping requirement).
        for f in nc.m.functions:
            for blk in f.blocks:
                for ins in blk.instructions:
                    if isinstance(ins, mybir.InstDMAGatherAnt):
                        si = ins.sync_info
                        lane = None
                        if si is not None:
                            for u in si.on_update:
                                nm = u.ant_name or ""
                                if nm.startswith("DMASW"):
                                    lane = int(nm[5:].split("_")[0])
                        if lane is not None:
                            ins.queue_num = lane % 4
    return nc


# ---------------------------------------------------------------------------
# Entry point
# ---------------------------------------------------------------------------

def _block_diag_a(a_src, a_dst):
    h, c = a_src.shape
    F1 = h * c
    ab = np.zeros((F1, 2 * h), dtype=np.float32)
    for hd in range(h):
        ab[hd * c:(hd + 1) * c, hd] = a_src[hd]
        ab[hd * c:(hd + 1) * c, h + hd] = a_dst[hd]
    return ab


def prepare(x, edge_index, W1, a_src1, a_dst1, b1, W2, a_src2, a_dst2, b2):
    import ml_dtypes
    x = np.asarray(x, dtype=np.float32)
    edge_index = np.asarray(edge_index)
    n_nodes, f_in = x.shape
    h1, c1 = np.asarray(a_src1).shape
    ncls = np.asarray(W2).shape[1]

    plan = build_plan(edge_index, n_nodes)
    nc = build_program(plan, f_in, h1, c1, ncls)

    AB1 = _block_diag_a(np.asarray(a_src1, np.float32),
                        np.asarray(a_dst1, np.float32))
    W1f = np.asarray(W1, np.float32)
    W1a = np.concatenate([W1f, W1f @ AB1], axis=1).astype(ml_dtypes.bfloat16)
    A2 = np.concatenate([np.asarray(a_src2, np.float32).T,
                         np.asarray(a_dst2, np.float32).T], axis=1)
    common = {
        "W1a": W1a,
        "b1r": np.asarray(b1, np.float32).reshape(1, -1),
        "W2": np.ascontiguousarray(W2, np.float32),
        "W2T": np.ascontiguousarray(np.asarray(W2, np.float32).T),
        "A2": np.ascontiguousarray(A2),
        "b2r": np.asarray(b2, np.float32).reshape(1, -1),
    }
    in_maps = []
    for c in range(N_CORES):
        pl = plan.cores[c]
        im = dict(common)
        xs = np.zeros((plan.pos, f_in), dtype=np.float32)
        xs[pl.posq] = x[c * plan.shard:(c + 1) * plan.shard]
        im["xT"] = np.ascontiguousarray(xs.T).astype(ml_dtypes.bfloat16)
        im["gidx"] = pl.gidx
        in_maps.append(im)
    return plan, nc, in_maps, (n_nodes, ncls)


def finish(plan, shard_outs, n_nodes, ncls):
    out = np.empty((n_nodes, ncls), dtype=np.float32)
    for c in range(N_CORES):
        pl = plan.cores[c]
        out[c * plan.shard:(c + 1) * plan.shard] = shard_outs[c][pl.posq]
    return out


def kernel(x, edge_index, W1, a_src1, a_dst1, b1, W2, a_src2, a_dst2, b2,
           **run_kwargs):
    plan, nc, in_maps, (n_nodes, ncls) = prepare(
        x, edge_index, W1, a_src1, a_dst1, b1, W2, a_src2, a_dst2, b2)
    res = run_bass_kernel_spmd(nc, in_maps, core_ids=list(range(N_CORES)),
                               **run_kwargs)
    out = finish(plan, [res.results[c]["out"] for c in range(N_CORES)],
                 n_nodes, ncls)
    kernel.last_result = res
    return out
